# revision 47
# baseline (speedup 1.0000x reference)
"""Multi-head self-attention (B=2, T=2048, C=1024, H=16, causal, position bias)
on 8 Trainium2 NeuronCores.

Sharding: 2 heads per core (tensor parallel over heads), both batches on every
core. QKV projection computed per-core for its own head slice (x replicated,
pre-transposed on host). Attention fully per-core. Output projection is
token-sharded after an on-device AllToAll of the head-sharded attention
output; host concatenates the 8 token slices.

v2 numerics/structure:
 - scores K=65 fold: row 64 of kT = 1.0, row 64 of q8T = -m0_s (strided max).
 - position bias applied multiplicatively: E = exp(8*(bias - bias_max)) is
   precomputed on host in bf16 (causal-invalid entries are exact 0.0), and
   multiplied into exp(scores - m0_s) on the DVE at 2x bf16 rate. Any
   per-query shift cancels in the softmax normalization, so m0_s only needs
   to be within ~80 of the true row max: it is computed from stride-16
   subsampled keys (A-phase), which only lowers it (safe side), with the
   diagonal-block causal mask folded in via a second accumulate-matmul whose
   pad columns overwrite garbage (has_written semantics).
 - reciprocal via reciprocal_approx_fast on a [4,512] batch per span.
 - v-transpose (v2 layout) built lazily per span to fill PE gaps.
 - output projection of span Q-1 issued after span Q's B-phase so the
   AllToAll overlaps a full span of compute.
"""
import numpy as np
import ml_dtypes

import concourse.bass as bass
import concourse.mybir as mybir
import concourse.tile as tile
from concourse import bacc
from concourse._compat import get_trn_type
from concourse.bass_utils import run_bass_kernel_spmd

F32 = mybir.dt.float32
BF16 = mybir.dt.bfloat16
F16 = mybir.dt.float16
AF = mybir.ActivationFunctionType

N_CORES = 8
B = 2
T = 2048
C = 1024
H = 16
D = 64
HPC = H // N_CORES        # heads per core = 2
QS = 512                  # query span (layout B free dim)
NSPAN = T // QS           # 4 spans per (b, head)
# A-phase key subsample stride per query span: early rows need dense max
# (few valid keys -> subsampled max can miss by >100), later rows have
# >=128 strided samples so the miss is bounded by ~30 (audited on host).
SPAN_STRIDE = [1, 4, 8, 16]
MPOFF = {4: 0, 8: 128, 16: 192}   # maskpadS column offsets per stride
NEG = -1.0e9

_CACHE = {}


def _build():
    nc = bacc.Bacc(get_trn_type() or "TRN2", target_bir_lowering=False,
                   debug=False, num_devices=N_CORES)

    # ---- per-core DRAM parameters (contents differ per core) ----
    xT = nc.declare_dram_parameter("xT", [B, C, T], F16, isOutput=False)           # x transposed
    wqkvT = nc.declare_dram_parameter("wqkvT", [C, 3 * 128], F16, isOutput=False)  # [in, q8|k|v]
    ebias = nc.declare_dram_parameter("ebias", [HPC, T, T], BF16, isOutput=False)  # exp(8(b-bmax)), [key, query]
    maskpadS = nc.declare_dram_parameter("maskpadS", [128, 224], BF16, isOutput=False)
    maskA = nc.declare_dram_parameter("maskA", [128, 128], BF16, isOutput=False)   # strict-upper -1e9 ([q,k])
    maskAT = nc.declare_dram_parameter("maskAT", [128, 128], BF16, isOutput=False)  # strict-lower -1e9 ([k,q])
    negb = nc.declare_dram_parameter("negb", [128, 384], BF16, isOutput=False)     # all -1e9
    id16 = nc.declare_dram_parameter("id16", [128, 128], BF16, isOutput=False)
    wprojT = nc.declare_dram_parameter("wprojT", [C, C], F16, isOutput=False)      # W_proj.T
    id_f = nc.declare_dram_parameter("id_f", [128, 128], F32, isOutput=False)      # identity (transpose)
    id64x2b = nc.declare_dram_parameter("id64x2b", [128, 64], BF16, isOutput=False)  # [I64; I64]
    ones_col = nc.declare_dram_parameter("ones_col", [128, 16], BF16, isOutput=False)  # all 1.0
    ones_row = nc.declare_dram_parameter("ones_row", [1, T], F16, isOutput=False)
    out = nc.declare_dram_parameter("out", [T * B // N_CORES, C], F32, isOutput=True)

    with tile.TileContext(nc) as tc:
        with (
            tc.tile_pool(name="consts", bufs=1) as consts,
            tc.tile_pool(name="wq", bufs=1) as wq_pool,
            tc.tile_pool(name="qkv", bufs=1) as qkv_pool,
            tc.tile_pool(name="stream", bufs=8) as stream,
            tc.tile_pool(name="obp", bufs=2) as obp,
            tc.tile_pool(name="bias", bufs=6) as bias_pool,
            tc.tile_pool(name="ptile", bufs=3) as p_pool,
            tc.tile_pool(name="petile", bufs=3) as pe_pool,
            tc.tile_pool(name="yinp", bufs=1) as yinp,
            tc.tile_pool(name="stats", bufs=1) as stats,
            tc.tile_pool(name="ytile", bufs=1) as y_pool,
            tc.tile_pool(name="small", bufs=2) as small,
            tc.tile_pool(name="psA", bufs=3, space="PSUM") as psA,
            tc.tile_pool(name="psY", bufs=2, space="PSUM") as psY,
            tc.tile_pool(name="dram", bufs=1, space="DRAM") as dram,
        ):
            # ---------------- launch-skew absorber ----------------
            # Tiny AllReduce on the gpsimd queue only: cores rendezvous here
            # while their other engines run QKV, so the first real AllToAll
            # doesn't eat the multi-core launch skew.
            bar = dram.tile([1, 8], F32, tag="bar", name="bar")
            bars = small.tile([1, 8], F32, tag="bars")
            nc.gpsimd.memset(bars[:], 0.0)
            nc.sync.dma_start(bar[:], bars[:])
            nc.gpsimd.collective_compute(
                "AllReduce", mybir.AluOpType.add,
                replica_groups=[list(range(N_CORES))],
                ins=[bar[:].opt()], outs=[bar[:].opt()])

            # ---------------- constants ----------------
            idf_t = consts.tile([128, 128], F32, tag="idf")
            nc.sync.dma_start(idf_t[:], id_f[:])
            id16_t = consts.tile([128, 128], BF16, tag="id16")
            nc.sync.dma_start(id16_t[:], id16[:])
            maskpad_t = consts.tile([128, 224], BF16, tag="maskpad")
            nc.sync.dma_start(maskpad_t[:], maskpadS[:])
            maskA_t = consts.tile([128, 128], BF16, tag="maskA")
            nc.sync.dma_start(maskA_t[:], maskA[:])
            maskAT_t = consts.tile([128, 128], BF16, tag="maskAT")
            nc.sync.dma_start(maskAT_t[:], maskAT[:])
            negb_t = consts.tile([128, 384], BF16, tag="negb")
            nc.sync.dma_start(negb_t[:], negb[:])
            id64b_t = consts.tile([128, 64], BF16, tag="id64b")
            nc.sync.dma_start(id64b_t[:], id64x2b[:])

            wqkv_t = wq_pool.tile([128, 8 * 384], F16, tag="wqkv")
            for kk in range(8):
                nc.sync.dma_start(wqkv_t[:, kk * 384:(kk + 1) * 384],
                                  wqkvT[kk * 128:(kk + 1) * 128, :])

            # ---------------- phase 1: QKV projection ----------------
            # q8T/kT per (b, head): [65, 2048]; row 64: q8T = -m0_s (per span),
            # kT = 1.0 (host). vT per b: [128 (2 heads), 2048].
            q8T = [[qkv_pool.tile([65, T], F16, tag=f"q8T{b}{j}", name=f"q8T{b}{j}")
                    for j in range(HPC)] for b in range(B)]
            kTt = [[qkv_pool.tile([65, T], F16, tag=f"kT{b}{j}", name=f"kT{b}{j}")
                    for j in range(HPC)] for b in range(B)]
            vTt = [qkv_pool.tile([128, T], BF16, tag=f"vT{b}", name=f"vT{b}")
                   for b in range(B)]
            for b in range(B):
                for j in range(HPC):
                    nc.sync.dma_start(kTt[b][j][64:65, :], ones_row[:, :])
            for b in range(B):
                # full-width x tiles, shared by both tp passes (halves the
                # DMA count and removes the second pass's DMA wait)
                xs8 = []
                for kk in range(8):
                    xs = stream.tile([128, 2048], F16, tag="xs")
                    nc.sync.dma_start(xs[:], xT[b, kk * 128:(kk + 1) * 128, :])
                    xs8.append(xs)
                for tp in range(2):
                    ps_m = [psA.tile([128, 1024], F32, tag="ps", name=f"psm{m_}")
                            for m_ in range(3)]
                    for kk in range(8):
                        for m in range(3):
                            for u in range(2):
                                nc.tensor.matmul(
                                    ps_m[m][:, u * 512:(u + 1) * 512],
                                    wqkv_t[:, kk * 384 + m * 128: kk * 384 + (m + 1) * 128],
                                    xs8[kk][:, tp * 1024 + u * 512:tp * 1024 + (u + 1) * 512],
                                    start=(kk == 0), stop=(kk == 7))
                    cols = slice(tp * 1024, (tp + 1) * 1024)
                    for j in range(HPC):
                        nc.vector.tensor_copy(q8T[b][j][0:64, cols],
                                              ps_m[0][64 * j:64 * (j + 1), :])
                        nc.scalar.copy(kTt[b][j][0:64, cols],
                                       ps_m[1][64 * j:64 * (j + 1), :])
                    nc.scalar.copy(vTt[b][:, cols], ps_m[2][:, :])

            # strided key snapshots for the A-phase max (strides 4/8/16)
            kTs = {}
            for S in (4, 8, 16):
                kTs[S] = [[qkv_pool.tile([64, T // S], F16, tag=f"kTs{S}{b}{j}",
                                         name=f"kTs{S}{b}{j}")
                           for j in range(HPC)] for b in range(B)]
            for b in range(B):
                for j in range(HPC):
                    nc.vector.tensor_copy(kTs[4][b][j][:, :],
                                          kTt[b][j][0:64, 0:T:4])
                    nc.vector.tensor_copy(kTs[8][b][j][:, :],
                                          kTs[4][b][j][:, 0:T // 4:2])
                    nc.vector.tensor_copy(kTs[16][b][j][:, :],
                                          kTs[8][b][j][:, 0:T // 8:2])

            # v token-major layout tiles (built lazily per span)
            v2 = [[y_pool.tile([128, 16 * 65], BF16, tag=f"v2_{b}{j}", name=f"v2_{b}{j}")
                   for j in range(HPC)] for b in range(B)]
            for b in range(B):
                for j in range(HPC):
                    nc.sync.dma_start(v2[b][j][:, 64::65], ones_col[:, :])

            # ---------------- phase 2: attention ----------------
            a2a_in = [dram.tile([8, 128, 128], F16, tag=f"a2a_in{q_}",
                                name=f"a2a_in{q_}") for q_ in range(NSPAN)]
            a2a_out = [dram.tile([8, 128, 128], F16, tag=f"a2a_out{q_}",
                                 name=f"a2a_out{q_}") for q_ in range(NSPAN)]
            # final span ships per-head halves so the tail AllToAll overlaps
            # the second head's normalize
            a2a_in3 = [dram.tile([8, 64, 128], F16, tag=f"a2a_in3{j_}",
                                 name=f"a2a_in3{j_}") for j_ in range(HPC)]
            a2a_out3 = [dram.tile([8, 64, 128], F16, tag=f"a2a_out3{j_}",
                                  name=f"a2a_out3{j_}") for j_ in range(HPC)]
            yin = [yinp.tile([128, 512], F16, tag=f"yin{r}", name=f"yin{r}")
                   for r in range(8)]
            wproj_t = wq_pool.tile([128, 8 * 1024], F16, tag="wproj")
            for r in range(8):
                nc.sync.dma_start(wproj_t[:, r * 1024:(r + 1) * 1024],
                                  wprojT[r * 128:(r + 1) * 128, :])

            def proj_pass(tt, split=False):
                if split:
                    for r in range(8):
                        nc.sync.dma_start(yin[r][0:64, tt * 128:(tt + 1) * 128],
                                          a2a_out3[0][r])
                        nc.sync.dma_start(yin[r][64:128, tt * 128:(tt + 1) * 128],
                                          a2a_out3[1][r])
                else:
                    for r in range(8):
                        nc.sync.dma_start(yin[r][:, tt * 128:(tt + 1) * 128],
                                          a2a_out[tt][r])
                pp = psA.tile([128, 1024], F32, tag="ps", name=f"pp{tt}")
                for oc in range(2):
                    for r in range(8):
                        nc.tensor.matmul(
                            pp[:, oc * 512:(oc + 1) * 512],
                            yin[r][:, tt * 128:(tt + 1) * 128],
                            wproj_t[:, r * 1024 + oc * 512: r * 1024 + (oc + 1) * 512],
                            start=(r == 0), stop=(r == 7))
                ob = obp.tile([128, 1024], F32, tag="ob")
                nc.scalar.copy(ob[:], pp[:])
                nc.sync.dma_start(out[tt * 128:(tt + 1) * 128, :], ob[:])

            for Q in range(NSPAN):
                # ---- A-phase: -max(8 q.k) over (strided) keys -> q8T row 64 ----
                S = SPAN_STRIDE[Q]
                NKS = 128 // S
                WMAX = (4 * Q + 4) * NKS
                for b in range(B):
                    for j in range(HPC):
                        mneg = stats.tile([128, 4], F32, tag=f"mneg{b}{j}",
                                          name=f"mneg{b}{j}")
                        if Q == 0:
                            # dense max for the first span's short rows
                            for ii in range(4):
                                n = (ii + 1) * 128
                                pa0 = psA.tile([128, 512], F32, tag="ps",
                                               name="pa0")
                                nc.tensor.matmul(
                                    pa0[:, 0:n],
                                    q8T[b][j][0:64, ii * 128:(ii + 1) * 128],
                                    kTt[b][j][0:64, 0:n],
                                    start=True, stop=False)
                                nc.tensor.matmul(
                                    pa0[:, ii * 128:n],
                                    id16_t[:], maskA_t[:],
                                    start=False, stop=True)
                                nc.vector.tensor_reduce(
                                    mneg[:, ii:ii + 1], pa0[:, 0:n],
                                    axis=mybir.AxisListType.X,
                                    op=mybir.AluOpType.max, negate=True)
                        else:
                            # two 1-bank tiles: start=True only clears the
                            # bank it writes, so each 2-row group gets its
                            # own accumulation group
                            for g in range(2):
                                pa = psA.tile([128, 2, WMAX], F32, tag="ps",
                                              name=f"pa{g}")
                                for r_ in range(2):
                                    ii = 2 * g + r_
                                    i = 4 * Q + ii
                                    ns = (i + 1) * NKS
                                    nc.tensor.matmul(
                                        pa[:, r_, 0:ns],
                                        q8T[b][j][0:64, i * 128:(i + 1) * 128],
                                        kTs[S][b][j][:, 0:ns],
                                        start=(r_ == 0), stop=False)
                                    # diag mask + -1e9 pad; pad cols are
                                    # overwritten via has_written semantics
                                    nc.tensor.matmul(
                                        pa[:, r_, i * NKS:WMAX],
                                        id16_t[:],
                                        maskpad_t[:, MPOFF[S]:MPOFF[S] + WMAX - i * NKS],
                                        start=False, stop=(r_ == 1))
                                nc.vector.tensor_reduce(
                                    mneg[:, 2 * g:2 * g + 2], pa[:, :, :],
                                    axis=mybir.AxisListType.X,
                                    op=mybir.AluOpType.max, negate=True)
                        tp = psA.tile([4, 128], F32, tag="ps", name="tp")
                        nc.tensor.transpose(tp[0:4, 0:128], mneg[:], idf_t[:])
                        mtr = small.tile([4, 128], F32, tag="mtr")
                        nc.scalar.copy(mtr[:], tp[0:4, 0:128])
                        nc.gpsimd.dma_start(
                            q8T[b][j][64:65, Q * 512:(Q + 1) * 512]
                            .rearrange("o (t p) -> o t p", t=4),
                            mtr[:])

                # ---- v2 lazy build: transpose this span's 4 new kt tiles ----
                for b in range(B):
                    for j in range(HPC):
                        pv = psA.tile([128, 4, 64], BF16, tag="ps", name="pv")
                        for c in range(4):
                            kt = 4 * Q + c
                            nc.tensor.transpose(
                                pv[:, c, :],
                                vTt[b][64 * j:64 * (j + 1),
                                       kt * 128:(kt + 1) * 128],
                                id64b_t[64 * j:64 * (j + 1), :])
                        nc.scalar.copy(
                            v2[b][j][:, 4 * Q * 65:(4 * Q + 4) * 65].rearrange(
                                "p (t c) -> p t c", t=4)[:, :, 0:64],
                            pv[:, :, :])

                # ---- B-phase: scores^T (K=65 folds -m0s), exp, *E, AV ----
                lst = [stats.tile([1, 512], F32, tag=f"lst{u}", name=f"lst{u}")
                       for u in range(4)]
                ysb = [stats.tile([64, 512], F32, tag=f"ysb{u}", name=f"ysb{u}")
                       for u in range(4)]
                for j in range(HPC):
                    pY = {}
                    for b in range(B):
                        pY[b] = psY.tile([128, 512], F32, tag="psY",
                                         name=f"pY{b}{j}")
                    for kt2 in range(0, 4 * Q + 4, 2):
                        etp = bias_pool.tile([128, 1024], BF16, tag="ebias",
                                             name="etp")
                        for u_ in range(2):
                            nc.sync.dma_start(
                                etp[:, u_ * 512:(u_ + 1) * 512],
                                ebias[j, (kt2 + u_) * 128:(kt2 + u_ + 1) * 128,
                                      Q * 512:(Q + 1) * 512])
                        # diagonal pairs with c>=2 are mostly masked: compute
                        # only the valid query range (stale pt cols are zeroed
                        # by the multiplicative E mask).
                        c2 = kt2 - 4 * Q
                        trim = c2 >= 2
                        for b in range(B):
                            pb = psA.tile([128, 1024], F32, tag="ps")
                            for u in range(2):
                                c = c2 + u
                                qo = c * 128 if trim else 0
                                nc.tensor.matmul(
                                    pb[:, u * 512 + qo:(u + 1) * 512],
                                    kTt[b][j][:, (kt2 + u) * 128:(kt2 + u + 1) * 128],
                                    q8T[b][j][:, Q * 512 + qo:(Q + 1) * 512],
                                    start=True, stop=(c < 0))
                                if c >= 0:
                                    if c > 0 and not trim:
                                        nc.tensor.matmul(
                                            pb[:, u * 512:u * 512 + c * 128],
                                            id16_t[:], negb_t[:, 0:c * 128],
                                            start=False, stop=False)
                                    nc.tensor.matmul(
                                        pb[:, u * 512 + c * 128:u * 512 + (c + 1) * 128],
                                        id16_t[:], maskAT_t[:],
                                        start=False, stop=True)
                            pt = p_pool.tile([128, 1024], BF16, tag="p")
                            if trim:
                                for u in range(2):
                                    qo = (c2 + u) * 128
                                    nc.scalar.activation(
                                        pt[:, u * 512 + qo:(u + 1) * 512],
                                        pb[:, u * 512 + qo:(u + 1) * 512],
                                        AF.Exp)
                            else:
                                nc.scalar.activation(pt[:], pb[:], AF.Exp)
                            ptE = pe_pool.tile([128, 1024], BF16, tag="pE")
                            nc.vector.tensor_tensor(
                                ptE[:], pt[:], etp[:], op=mybir.AluOpType.mult)
                            for u in range(2):
                                qo = (c2 + u) * 128 if trim else 0
                                nc.tensor.matmul(
                                    pY[b][0:65, qo:],
                                    v2[b][j][:, (kt2 + u) * 65:(kt2 + u + 1) * 65],
                                    ptE[:, u * 512 + qo:(u + 1) * 512],
                                    start=(kt2 + u == 0),
                                    stop=(kt2 + u == 4 * Q + 3))

                    # evacuate psY + normalize + scatter for this head
                    last = Q == NSPAN - 1
                    for b in range(B):
                        un = 2 * j + b
                        nc.scalar.copy(lst[un][:, :], pY[b][64:65, :])
                        nc.vector.tensor_copy(ysb[un][:, :], pY[b][0:64, :])
                        linv = stats.tile([1, 512], F32, tag=f"linv{un}",
                                          name=f"linv{un}")
                        nc.vector.reciprocal_approx_fast(linv[:], lst[un][:])
                        linb = small.tile([64, 512], F32, tag="linb")
                        nc.gpsimd.partition_broadcast(
                            linb[:], linv[:, :], channels=64)
                        ytmp = small.tile([64, 512], F16, tag="ytmp")
                        nc.vector.tensor_tensor(
                            ytmp[:], ysb[un][:], linb[:],
                            op=mybir.AluOpType.mult)
                        if last:
                            dst = a2a_in3[j][:, :, 64 * b:64 * (b + 1)]
                        else:
                            dst = a2a_in[Q][:, 64 * j:64 * (j + 1),
                                            64 * b:64 * (b + 1)]
                        nc.sync.dma_start(
                            dst.rearrange("r c i -> c r i"),
                            ytmp[:].rearrange("c (r i) -> c r i", r=8))
                    if last:
                        nc.gpsimd.collective_compute(
                            "AllToAll", mybir.AluOpType.bypass,
                            replica_groups=[list(range(N_CORES))],
                            ins=[a2a_in3[j].opt()], outs=[a2a_out3[j].opt()])

                # ---- projection of previous span (AllToAll has landed) ----
                if Q > 0:
                    proj_pass(Q - 1)

                if not last:
                    nc.gpsimd.collective_compute(
                        "AllToAll", mybir.AluOpType.bypass,
                        replica_groups=[list(range(N_CORES))],
                        ins=[a2a_in[Q].opt()], outs=[a2a_out[Q].opt()])

            # ---------------- phase 4: final slice ----------------
            proj_pass(NSPAN - 1, split=True)

    nc.finalize()
    return nc


def _prep_inputs(x, position_bias, W_attn, W_proj):
    """Host-side shard/layout prep. Returns in_maps for the 8 cores."""
    x = np.asarray(x, np.float32)
    pb = np.asarray(position_bias, np.float32)[0]          # [H, T, T]
    W_attn = np.asarray(W_attn, np.float32)
    W_proj = np.asarray(W_proj, np.float32)

    xT = np.ascontiguousarray(x.transpose(0, 2, 1)).astype(np.float16)  # [B, C, T]
    wprojT = np.ascontiguousarray(W_proj.T).astype(np.float16)     # [in, out]
    id_f = np.eye(128, dtype=np.float32)
    ones_col_np = np.ones((128, 16), ml_dtypes.bfloat16)
    id64x2_np = np.vstack([np.eye(64, dtype=np.float32)] * 2)
    ones_row_np = np.ones((1, T), np.float16)

    # A-phase diag-mask + pad tiles (per stride): rows=query-in-block,
    # col c < NKS: -1e9 where key (S*c) > query; col >= NKS: -1e9 pad.
    qv = np.arange(128)[:, None]
    mps = []
    for S, w in ((4, 128), (8, 64), (16, 32)):
        nks = 128 // S
        mp = np.zeros((128, w), np.float32)
        cv = np.arange(w)[None, :]
        mp[(cv < nks) & (S * cv > qv)] = NEG
        mp[:, nks:] = NEG
        mps.append(mp)
    maskpad_np = np.concatenate(mps, 1).astype(ml_dtypes.bfloat16)
    maskA_np = np.triu(np.full((128, 128), NEG, np.float32), 1).astype(ml_dtypes.bfloat16)
    maskAT_np = np.tril(np.full((128, 128), NEG, np.float32), -1).astype(ml_dtypes.bfloat16)
    negb_np = np.full((128, 384), NEG, np.float32).astype(ml_dtypes.bfloat16)
    id16_np = np.eye(128, dtype=np.float32).astype(ml_dtypes.bfloat16)

    tril = np.tril(np.ones((T, T), dtype=bool))
    in_maps = []
    for c in range(N_CORES):
        wq = W_attn[128 * c:128 * (c + 1), :] * 8.0
        wk = W_attn[C + 128 * c:C + 128 * (c + 1), :]
        wv = W_attn[2 * C + 128 * c:2 * C + 128 * (c + 1), :]
        wqkvT = np.ascontiguousarray(np.concatenate([wq, wk, wv], 0).T).astype(np.float16)
        eb = np.empty((HPC, T, T), ml_dtypes.bfloat16)
        for j in range(HPC):
            h = HPC * c + j
            bh = pb[h]
            bmax = float(bh[tril].max())
            # PAD=12 keeps the f32 AV/denominator accumulation away from
            # overflow (max exp arg ~86 + ln(2048*|v|) would pass 88.7);
            # it costs 12 on the underflow gap (64.7+12 < 85, audited).
            ebj = np.exp(8.0 * (bh.T - bmax) - 12.0)       # [key, query]
            ebj[~tril.T] = 0.0                             # key > query -> 0
            eb[j] = ebj.astype(ml_dtypes.bfloat16)
        in_maps.append({
            "xT": xT, "wqkvT": wqkvT, "ebias": np.ascontiguousarray(eb),
            "wprojT": wprojT, "id_f": id_f,
            "maskpadS": maskpad_np, "maskA": maskA_np, "maskAT": maskAT_np,
            "negb": negb_np,
            "id16": id16_np,
            "id64x2b": id64x2_np.astype(ml_dtypes.bfloat16),
            "ones_col": ones_col_np,
            "ones_row": ones_row_np,
        })
    return in_maps


def kernel(x, position_bias, W_attn, W_proj, _trace=False, _tmpdir=None):
    if "nc" not in _CACHE:
        _CACHE["nc"] = _build()
    nc = _CACHE["nc"]
    in_maps = _prep_inputs(x, position_bias, W_attn, W_proj)
    res = run_bass_kernel_spmd(nc, in_maps, list(range(N_CORES)),
                               trace=_trace, tmpdir=_tmpdir)
    if _trace:
        _CACHE["exec_time_ns"] = res.exec_time_ns
    out_full = np.empty((B, T, C), np.float32)
    for c in range(N_CORES):
        r = res.results[c]["out"].reshape(NSPAN, B, 64, C)
        for b in range(B):
            for Qs in range(NSPAN):
                out_full[b, Qs * 512 + 64 * c: Qs * 512 + 64 * (c + 1)] = r[Qs, b]
    return out_full


# revision 52
# speedup vs baseline: 1.1204x; 1.1204x over previous
"""Multi-head self-attention (B=2, T=2048, C=1024, H=16, causal, position bias)
on 8 Trainium2 NeuronCores.

Sharding: 2 heads per core (tensor parallel over heads), both batches on every
core. QKV projection computed per-core for its own head slice (x replicated,
pre-transposed on host). Attention fully per-core. Output projection is
token-sharded after an on-device AllToAll of the head-sharded attention
output; host concatenates the 8 token slices.

v2 numerics/structure:
 - scores K=65 fold: row 64 of kT = 1.0, row 64 of q8T = -m0_s (strided max).
 - position bias applied multiplicatively: E = exp(8*(bias - bias_max)) is
   precomputed on host in bf16 (causal-invalid entries are exact 0.0), and
   multiplied into exp(scores - m0_s) on the DVE at 2x bf16 rate. Any
   per-query shift cancels in the softmax normalization, so m0_s only needs
   to be within ~80 of the true row max: it is computed from stride-16
   subsampled keys (A-phase), which only lowers it (safe side), with the
   diagonal-block causal mask folded in via a second accumulate-matmul whose
   pad columns overwrite garbage (has_written semantics).
 - reciprocal via reciprocal_approx_fast on a [4,512] batch per span.
 - v-transpose (v2 layout) built lazily per span to fill PE gaps.
 - output projection of span Q-1 issued after span Q's B-phase so the
   AllToAll overlaps a full span of compute.
"""
import numpy as np
import ml_dtypes

import concourse.bass as bass
import concourse.mybir as mybir
import concourse.tile as tile
from concourse import bacc
from concourse._compat import get_trn_type
from concourse.bass_utils import run_bass_kernel_spmd

F32 = mybir.dt.float32
BF16 = mybir.dt.bfloat16
F16 = mybir.dt.float16
AF = mybir.ActivationFunctionType

N_CORES = 8
B = 2
T = 2048
C = 1024
H = 16
D = 64
HPC = H // N_CORES        # heads per core = 2
QS = 512                  # query span (layout B free dim)
NSPAN = T // QS           # 4 spans per (b, head)
# A-phase key subsample stride per query span: early rows need dense max
# (few valid keys -> subsampled max can miss by >100), later rows have
# >=128 strided samples so the miss is bounded by ~30 (audited on host).
SPAN_STRIDE = [1, 4, 8, 16]
MPOFF = {4: 0, 8: 128, 16: 192}   # maskpadS column offsets per stride
NEG = -1.0e9

_CACHE = {}


def _build():
    nc = bacc.Bacc(get_trn_type() or "TRN2", target_bir_lowering=False,
                   debug=False, num_devices=N_CORES)

    # ---- per-core DRAM parameters (contents differ per core) ----
    xT = nc.declare_dram_parameter("xT", [B, C, T], F16, isOutput=False)           # x transposed
    wqkvT = nc.declare_dram_parameter("wqkvT", [C, 3 * 128], F16, isOutput=False)  # [in, q8|k|v]
    ebias = nc.declare_dram_parameter("ebias", [HPC, T, T], BF16, isOutput=False)  # exp(8(b-bmax)), [key, query]
    maskpadS = nc.declare_dram_parameter("maskpadS", [128, 224], BF16, isOutput=False)
    maskA = nc.declare_dram_parameter("maskA", [128, 128], BF16, isOutput=False)   # strict-upper -1e9 ([q,k])
    maskAT = nc.declare_dram_parameter("maskAT", [128, 128], BF16, isOutput=False)  # strict-lower -1e9 ([k,q])
    negb = nc.declare_dram_parameter("negb", [128, 384], BF16, isOutput=False)     # all -1e9
    id16 = nc.declare_dram_parameter("id16", [128, 128], BF16, isOutput=False)
    wprojT = nc.declare_dram_parameter("wprojT", [C, C], F16, isOutput=False)      # W_proj.T
    id_f = nc.declare_dram_parameter("id_f", [128, 128], F32, isOutput=False)      # identity (transpose)
    id64x2b = nc.declare_dram_parameter("id64x2b", [128, 64], BF16, isOutput=False)  # [I64; I64]
    ones_col = nc.declare_dram_parameter("ones_col", [128, 16], BF16, isOutput=False)  # all 1.0
    ones_row = nc.declare_dram_parameter("ones_row", [1, T], F16, isOutput=False)
    out = nc.declare_dram_parameter("out", [T * B // N_CORES, C], F32, isOutput=True)

    with tile.TileContext(nc) as tc:
        with (
            tc.tile_pool(name="consts", bufs=1) as consts,
            tc.tile_pool(name="wq", bufs=1) as wq_pool,
            tc.tile_pool(name="qkv", bufs=1) as qkv_pool,
            tc.tile_pool(name="stream", bufs=8) as stream,
            tc.tile_pool(name="obp", bufs=2) as obp,
            tc.tile_pool(name="bias", bufs=6) as bias_pool,
            tc.tile_pool(name="ptile", bufs=3) as p_pool,
            tc.tile_pool(name="petile", bufs=3) as pe_pool,
            tc.tile_pool(name="yinp", bufs=1) as yinp,
            tc.tile_pool(name="stats", bufs=1) as stats,
            tc.tile_pool(name="ytile", bufs=1) as y_pool,
            tc.tile_pool(name="small", bufs=2) as small,
            tc.tile_pool(name="psA", bufs=3, space="PSUM") as psA,
            tc.tile_pool(name="psY", bufs=2, space="PSUM") as psY,
            tc.tile_pool(name="dram", bufs=1, space="DRAM") as dram,
        ):
            # ---------------- launch-skew absorber ----------------
            # Tiny AllReduce on the gpsimd queue only: cores rendezvous here
            # while their other engines run QKV, so the first real AllToAll
            # doesn't eat the multi-core launch skew.
            bar = dram.tile([1, 8], F32, tag="bar", name="bar")
            bars = small.tile([1, 8], F32, tag="bars")
            nc.gpsimd.memset(bars[:], 0.0)
            nc.sync.dma_start(bar[:], bars[:])
            nc.gpsimd.collective_compute(
                "AllReduce", mybir.AluOpType.add,
                replica_groups=[list(range(N_CORES))],
                ins=[bar[:].opt()], outs=[bar[:].opt()])

            # ---------------- constants ----------------
            idf_t = consts.tile([128, 128], F32, tag="idf")
            nc.sync.dma_start(idf_t[:], id_f[:])
            id16_t = consts.tile([128, 128], BF16, tag="id16")
            nc.sync.dma_start(id16_t[:], id16[:])
            maskpad_t = consts.tile([128, 224], BF16, tag="maskpad")
            nc.sync.dma_start(maskpad_t[:], maskpadS[:])
            maskA_t = consts.tile([128, 128], BF16, tag="maskA")
            nc.sync.dma_start(maskA_t[:], maskA[:])
            maskAT_t = consts.tile([128, 128], BF16, tag="maskAT")
            nc.sync.dma_start(maskAT_t[:], maskAT[:])
            negb_t = consts.tile([128, 384], BF16, tag="negb")
            nc.sync.dma_start(negb_t[:], negb[:])
            id64b_t = consts.tile([128, 64], BF16, tag="id64b")
            nc.sync.dma_start(id64b_t[:], id64x2b[:])

            wqkv_t = wq_pool.tile([128, 8 * 384], F16, tag="wqkv")
            for kk in range(8):
                nc.sync.dma_start(wqkv_t[:, kk * 384:(kk + 1) * 384],
                                  wqkvT[kk * 128:(kk + 1) * 128, :])

            # ---------------- phase 1: QKV projection ----------------
            # q8T/kT per (b, head): [65, 2048]; row 64: q8T = -m0_s (per span),
            # kT = 1.0 (host). vT per b: [128 (2 heads), 2048].
            q8T = [[qkv_pool.tile([65, T], F16, tag=f"q8T{b}{j}", name=f"q8T{b}{j}")
                    for j in range(HPC)] for b in range(B)]
            kTt = [[qkv_pool.tile([65, T], F16, tag=f"kT{b}{j}", name=f"kT{b}{j}")
                    for j in range(HPC)] for b in range(B)]
            vTt = [qkv_pool.tile([128, T], BF16, tag=f"vT{b}", name=f"vT{b}")
                   for b in range(B)]
            for b in range(B):
                for j in range(HPC):
                    nc.sync.dma_start(kTt[b][j][64:65, :], ones_row[:, :])
            for b in range(B):
                # full-width x tiles, shared by both tp passes (halves the
                # DMA count and removes the second pass's DMA wait)
                xs8 = []
                for kk in range(8):
                    xs = stream.tile([128, 2048], F16, tag="xs")
                    nc.sync.dma_start(xs[:], xT[b, kk * 128:(kk + 1) * 128, :])
                    xs8.append(xs)
                for tp in range(2):
                    ps_m = [psA.tile([128, 1024], F32, tag="ps", name=f"psm{m_}")
                            for m_ in range(3)]
                    for kk in range(8):
                        for m in range(3):
                            for u in range(2):
                                nc.tensor.matmul(
                                    ps_m[m][:, u * 512:(u + 1) * 512],
                                    wqkv_t[:, kk * 384 + m * 128: kk * 384 + (m + 1) * 128],
                                    xs8[kk][:, tp * 1024 + u * 512:tp * 1024 + (u + 1) * 512],
                                    start=(kk == 0), stop=(kk == 7))
                    cols = slice(tp * 1024, (tp + 1) * 1024)
                    for j in range(HPC):
                        nc.vector.tensor_copy(q8T[b][j][0:64, cols],
                                              ps_m[0][64 * j:64 * (j + 1), :])
                        nc.scalar.copy(kTt[b][j][0:64, cols],
                                       ps_m[1][64 * j:64 * (j + 1), :])
                    nc.scalar.copy(vTt[b][:, cols], ps_m[2][:, :])

            # strided key snapshots for the A-phase max (strides 4/8/16)
            kTs = {}
            for S in (4, 8, 16):
                kTs[S] = [[qkv_pool.tile([64, T // S], F16, tag=f"kTs{S}{b}{j}",
                                         name=f"kTs{S}{b}{j}")
                           for j in range(HPC)] for b in range(B)]
            for b in range(B):
                for j in range(HPC):
                    nc.vector.tensor_copy(kTs[4][b][j][:, :],
                                          kTt[b][j][0:64, 0:T:4])
                    nc.vector.tensor_copy(kTs[8][b][j][:, :],
                                          kTs[4][b][j][:, 0:T // 4:2])
                    nc.vector.tensor_copy(kTs[16][b][j][:, :],
                                          kTs[8][b][j][:, 0:T // 8:2])

            # v token-major layout tiles (built lazily per span)
            v2 = [[y_pool.tile([128, 16 * 65], BF16, tag=f"v2_{b}{j}", name=f"v2_{b}{j}")
                   for j in range(HPC)] for b in range(B)]
            for b in range(B):
                for j in range(HPC):
                    nc.sync.dma_start(v2[b][j][:, 64::65], ones_col[:, :])

            # ---------------- phase 2: attention ----------------
            a2a_in = [dram.tile([8, 128, 128], F16, tag=f"a2a_in{q_}",
                                name=f"a2a_in{q_}") for q_ in range(NSPAN)]
            a2a_out = [dram.tile([8, 128, 128], F16, tag=f"a2a_out{q_}",
                                 name=f"a2a_out{q_}") for q_ in range(NSPAN)]
            # final span ships per-head halves so the tail AllToAll overlaps
            # the second head's normalize
            a2a_in3 = [dram.tile([8, 64, 128], F16, tag=f"a2a_in3{j_}",
                                 name=f"a2a_in3{j_}") for j_ in range(HPC)]
            a2a_out3 = [dram.tile([8, 64, 128], F16, tag=f"a2a_out3{j_}",
                                  name=f"a2a_out3{j_}") for j_ in range(HPC)]
            yin = [yinp.tile([128, 512], F16, tag=f"yin{r}", name=f"yin{r}")
                   for r in range(8)]
            wproj_t = wq_pool.tile([128, 8 * 1024], F16, tag="wproj")
            for r in range(8):
                nc.sync.dma_start(wproj_t[:, r * 1024:(r + 1) * 1024],
                                  wprojT[r * 128:(r + 1) * 128, :])

            def proj_pass(tt, split=False):
                if split:
                    for r in range(8):
                        nc.sync.dma_start(yin[r][0:64, tt * 128:(tt + 1) * 128],
                                          a2a_out3[0][r])
                        nc.sync.dma_start(yin[r][64:128, tt * 128:(tt + 1) * 128],
                                          a2a_out3[1][r])
                else:
                    for r in range(8):
                        nc.sync.dma_start(yin[r][:, tt * 128:(tt + 1) * 128],
                                          a2a_out[tt][r])
                pp = psA.tile([128, 1024], F32, tag="ps", name=f"pp{tt}")
                for oc in range(2):
                    for r in range(8):
                        nc.tensor.matmul(
                            pp[:, oc * 512:(oc + 1) * 512],
                            yin[r][:, tt * 128:(tt + 1) * 128],
                            wproj_t[:, r * 1024 + oc * 512: r * 1024 + (oc + 1) * 512],
                            start=(r == 0), stop=(r == 7))
                ob = obp.tile([128, 1024], F32, tag="ob")
                nc.scalar.copy(ob[:], pp[:])
                nc.sync.dma_start(out[tt * 128:(tt + 1) * 128, :], ob[:])

            def a_phase_unit(Q, b, j):
                """-max(8 q.k) over (strided) keys -> q8T row 64 for span Q."""
                S = SPAN_STRIDE[Q]
                NKS = 128 // S
                WMAX = (4 * Q + 4) * NKS
                mneg = stats.tile([128, 4], F32, tag=f"mneg{b}{j}",
                                  name=f"mneg{b}{j}")
                if Q == 0:
                    # dense max for the first span's short rows
                    for ii in range(4):
                        n = (ii + 1) * 128
                        pa0 = psA.tile([128, 512], F32, tag="ps", name="pa0")
                        nc.tensor.matmul(
                            pa0[:, 0:n],
                            q8T[b][j][0:64, ii * 128:(ii + 1) * 128],
                            kTt[b][j][0:64, 0:n],
                            start=True, stop=False)
                        nc.tensor.matmul(
                            pa0[:, ii * 128:n],
                            id16_t[:], maskA_t[:],
                            start=False, stop=True)
                        nc.vector.tensor_reduce(
                            mneg[:, ii:ii + 1], pa0[:, 0:n],
                            axis=mybir.AxisListType.X,
                            op=mybir.AluOpType.max, negate=True)
                else:
                    # two 1-bank tiles: start=True only clears the bank it
                    # writes, so each 2-row group is its own accum group
                    for g in range(2):
                        pa = psA.tile([128, 2, WMAX], F32, tag="ps",
                                      name=f"pa{g}")
                        for r_ in range(2):
                            ii = 2 * g + r_
                            i = 4 * Q + ii
                            ns = (i + 1) * NKS
                            nc.tensor.matmul(
                                pa[:, r_, 0:ns],
                                q8T[b][j][0:64, i * 128:(i + 1) * 128],
                                kTs[S][b][j][:, 0:ns],
                                start=(r_ == 0), stop=False)
                            # diag mask + -1e9 pad; pad cols overwritten
                            # via has_written semantics
                            nc.tensor.matmul(
                                pa[:, r_, i * NKS:WMAX],
                                id16_t[:],
                                maskpad_t[:, MPOFF[S]:MPOFF[S] + WMAX - i * NKS],
                                start=False, stop=(r_ == 1))
                        nc.vector.tensor_reduce(
                            mneg[:, 2 * g:2 * g + 2], pa[:, :, :],
                            axis=mybir.AxisListType.X,
                            op=mybir.AluOpType.max, negate=True)
                tp = psA.tile([4, 128], F32, tag="ps", name="tp")
                nc.tensor.transpose(tp[0:4, 0:128], mneg[:], idf_t[:])
                mtr = small.tile([4, 128], F32, tag="mtr")
                nc.scalar.copy(mtr[:], tp[0:4, 0:128])
                nc.gpsimd.dma_start(
                    q8T[b][j][64:65, Q * 512:(Q + 1) * 512]
                    .rearrange("o (t p) -> o t p", t=4),
                    mtr[:])

            def v2_unit(Q, b, j):
                """v token-major build for span Q's 4 new kt tiles."""
                pv = psA.tile([128, 4, 64], BF16, tag="ps", name="pv")
                for c in range(4):
                    kt = 4 * Q + c
                    nc.tensor.transpose(
                        pv[:, c, :],
                        vTt[b][64 * j:64 * (j + 1),
                               kt * 128:(kt + 1) * 128],
                        id64b_t[64 * j:64 * (j + 1), :])
                nc.scalar.copy(
                    v2[b][j][:, 4 * Q * 65:(4 * Q + 4) * 65].rearrange(
                        "p (t c) -> p t c", t=4)[:, :, 0:64],
                    pv[:, :, :])

            # span 0 prologue: its A-phase + v layout must precede B(0)
            for b in range(B):
                for j in range(HPC):
                    a_phase_unit(0, b, j)
                    v2_unit(0, b, j)

            for Q in range(NSPAN):
                # filler PE work interleaved into the B-phase so the tensor
                # queue never head-of-line blocks on ACT/DVE round-trips:
                # next span's A-phase + v layout, previous span's projection.
                fillers = []
                if Q + 1 < NSPAN:
                    for b in range(B):
                        for j in range(HPC):
                            fillers.append(lambda b=b, j=j: a_phase_unit(Q + 1, b, j))
                            fillers.append(lambda b=b, j=j: v2_unit(Q + 1, b, j))
                if Q > 0:
                    fillers.append(lambda: proj_pass(Q - 1))
                niter = (2 * Q + 2) * HPC
                nfill = len(fillers)
                fstate = [0, 0]  # iterations done, fillers emitted

                # ---- B-phase: scores^T (K=65 folds -m0s), exp, *E, AV ----
                lst = [stats.tile([1, 512], F32, tag=f"lst{u}", name=f"lst{u}")
                       for u in range(4)]
                ysb = [stats.tile([64, 512], F32, tag=f"ysb{u}", name=f"ysb{u}")
                       for u in range(4)]
                for j in range(HPC):
                    pY = {}
                    for b in range(B):
                        pY[b] = psY.tile([128, 512], F32, tag="psY",
                                         name=f"pY{b}{j}")
                    for kt2 in range(0, 4 * Q + 4, 2):
                        etp = bias_pool.tile([128, 1024], BF16, tag="ebias",
                                             name="etp")
                        for u_ in range(2):
                            nc.sync.dma_start(
                                etp[:, u_ * 512:(u_ + 1) * 512],
                                ebias[j, (kt2 + u_) * 128:(kt2 + u_ + 1) * 128,
                                      Q * 512:(Q + 1) * 512])
                        for b in range(B):
                            pb = psA.tile([128, 1024], F32, tag="ps")
                            # diagonal-span tiles: fully-invalid query cols
                            # [0, c*128) get -1e9, diag block strict-lower.
                            for u in range(2):
                                cols = slice(u * 512, (u + 1) * 512)
                                c = kt2 + u - 4 * Q
                                nc.tensor.matmul(
                                    pb[:, cols],
                                    kTt[b][j][:, (kt2 + u) * 128:(kt2 + u + 1) * 128],
                                    q8T[b][j][:, Q * 512:(Q + 1) * 512],
                                    start=True, stop=(c < 0))
                                if c >= 0:
                                    if c > 0:
                                        nc.tensor.matmul(
                                            pb[:, u * 512:u * 512 + c * 128],
                                            id16_t[:], negb_t[:, 0:c * 128],
                                            start=False, stop=False)
                                    nc.tensor.matmul(
                                        pb[:, u * 512 + c * 128:u * 512 + (c + 1) * 128],
                                        id16_t[:], maskAT_t[:],
                                        start=False, stop=True)
                            pt = p_pool.tile([128, 1024], BF16, tag="p")
                            nc.scalar.activation(pt[:], pb[:], AF.Exp)
                            ptE = pe_pool.tile([128, 1024], BF16, tag="pE")
                            nc.vector.tensor_tensor(
                                ptE[:], pt[:], etp[:], op=mybir.AluOpType.mult)
                            for u in range(2):
                                nc.tensor.matmul(
                                    pY[b][0:65, :],
                                    v2[b][j][:, (kt2 + u) * 65:(kt2 + u + 1) * 65],
                                    ptE[:, u * 512:(u + 1) * 512],
                                    start=(kt2 + u == 0),
                                    stop=(kt2 + u == 4 * Q + 3))
                        # spread fillers evenly across the span's iterations
                        fstate[0] += 1
                        want = nfill * fstate[0] // niter
                        while fstate[1] < want and fillers:
                            fillers.pop(0)()
                            fstate[1] += 1

                    # evacuate psY + normalize + scatter for this head
                    last = Q == NSPAN - 1
                    for b in range(B):
                        un = 2 * j + b
                        nc.scalar.copy(lst[un][:, :], pY[b][64:65, :])
                        nc.vector.tensor_copy(ysb[un][:, :], pY[b][0:64, :])
                        linv = stats.tile([1, 512], F32, tag=f"linv{un}",
                                          name=f"linv{un}")
                        nc.vector.reciprocal_approx_fast(linv[:], lst[un][:])
                        linb = small.tile([64, 512], F32, tag="linb")
                        nc.gpsimd.partition_broadcast(
                            linb[:], linv[:, :], channels=64)
                        ytmp = small.tile([64, 512], F16, tag="ytmp")
                        nc.vector.tensor_tensor(
                            ytmp[:], ysb[un][:], linb[:],
                            op=mybir.AluOpType.mult)
                        if last:
                            dst = a2a_in3[j][:, :, 64 * b:64 * (b + 1)]
                        else:
                            dst = a2a_in[Q][:, 64 * j:64 * (j + 1),
                                            64 * b:64 * (b + 1)]
                        nc.sync.dma_start(
                            dst.rearrange("r c i -> c r i"),
                            ytmp[:].rearrange("c (r i) -> c r i", r=8))
                    if last:
                        nc.gpsimd.collective_compute(
                            "AllToAll", mybir.AluOpType.bypass,
                            replica_groups=[list(range(N_CORES))],
                            ins=[a2a_in3[j].opt()], outs=[a2a_out3[j].opt()])

                # drain any remaining fillers
                while fillers:
                    fillers.pop(0)()

                if not last:
                    nc.gpsimd.collective_compute(
                        "AllToAll", mybir.AluOpType.bypass,
                        replica_groups=[list(range(N_CORES))],
                        ins=[a2a_in[Q].opt()], outs=[a2a_out[Q].opt()])

            # ---------------- phase 4: final slice ----------------
            proj_pass(NSPAN - 1, split=True)

    nc.finalize()
    return nc


def _prep_inputs(x, position_bias, W_attn, W_proj):
    """Host-side shard/layout prep. Returns in_maps for the 8 cores."""
    x = np.asarray(x, np.float32)
    pb = np.asarray(position_bias, np.float32)[0]          # [H, T, T]
    W_attn = np.asarray(W_attn, np.float32)
    W_proj = np.asarray(W_proj, np.float32)

    xT = np.ascontiguousarray(x.transpose(0, 2, 1)).astype(np.float16)  # [B, C, T]
    wprojT = np.ascontiguousarray(W_proj.T).astype(np.float16)     # [in, out]
    id_f = np.eye(128, dtype=np.float32)
    ones_col_np = np.ones((128, 16), ml_dtypes.bfloat16)
    id64x2_np = np.vstack([np.eye(64, dtype=np.float32)] * 2)
    ones_row_np = np.ones((1, T), np.float16)

    # A-phase diag-mask + pad tiles (per stride): rows=query-in-block,
    # col c < NKS: -1e9 where key (S*c) > query; col >= NKS: -1e9 pad.
    qv = np.arange(128)[:, None]
    mps = []
    for S, w in ((4, 128), (8, 64), (16, 32)):
        nks = 128 // S
        mp = np.zeros((128, w), np.float32)
        cv = np.arange(w)[None, :]
        mp[(cv < nks) & (S * cv > qv)] = NEG
        mp[:, nks:] = NEG
        mps.append(mp)
    maskpad_np = np.concatenate(mps, 1).astype(ml_dtypes.bfloat16)
    maskA_np = np.triu(np.full((128, 128), NEG, np.float32), 1).astype(ml_dtypes.bfloat16)
    maskAT_np = np.tril(np.full((128, 128), NEG, np.float32), -1).astype(ml_dtypes.bfloat16)
    negb_np = np.full((128, 384), NEG, np.float32).astype(ml_dtypes.bfloat16)
    id16_np = np.eye(128, dtype=np.float32).astype(ml_dtypes.bfloat16)

    tril = np.tril(np.ones((T, T), dtype=bool))
    in_maps = []
    for c in range(N_CORES):
        wq = W_attn[128 * c:128 * (c + 1), :] * 8.0
        wk = W_attn[C + 128 * c:C + 128 * (c + 1), :]
        wv = W_attn[2 * C + 128 * c:2 * C + 128 * (c + 1), :]
        wqkvT = np.ascontiguousarray(np.concatenate([wq, wk, wv], 0).T).astype(np.float16)
        eb = np.empty((HPC, T, T), ml_dtypes.bfloat16)
        for j in range(HPC):
            h = HPC * c + j
            bh = pb[h]
            bmax = float(bh[tril].max())
            # PAD=12 keeps the f32 AV/denominator accumulation away from
            # overflow (max exp arg ~86 + ln(2048*|v|) would pass 88.7);
            # it costs 12 on the underflow gap (64.7+12 < 85, audited).
            ebj = np.exp(8.0 * (bh.T - bmax) - 12.0)       # [key, query]
            ebj[~tril.T] = 0.0                             # key > query -> 0
            eb[j] = ebj.astype(ml_dtypes.bfloat16)
        in_maps.append({
            "xT": xT, "wqkvT": wqkvT, "ebias": np.ascontiguousarray(eb),
            "wprojT": wprojT, "id_f": id_f,
            "maskpadS": maskpad_np, "maskA": maskA_np, "maskAT": maskAT_np,
            "negb": negb_np,
            "id16": id16_np,
            "id64x2b": id64x2_np.astype(ml_dtypes.bfloat16),
            "ones_col": ones_col_np,
            "ones_row": ones_row_np,
        })
    return in_maps


def kernel(x, position_bias, W_attn, W_proj, _trace=False, _tmpdir=None):
    if "nc" not in _CACHE:
        _CACHE["nc"] = _build()
    nc = _CACHE["nc"]
    in_maps = _prep_inputs(x, position_bias, W_attn, W_proj)
    res = run_bass_kernel_spmd(nc, in_maps, list(range(N_CORES)),
                               trace=_trace, tmpdir=_tmpdir)
    if _trace:
        _CACHE["exec_time_ns"] = res.exec_time_ns
    out_full = np.empty((B, T, C), np.float32)
    for c in range(N_CORES):
        r = res.results[c]["out"].reshape(NSPAN, B, 64, C)
        for b in range(B):
            for Qs in range(NSPAN):
                out_full[b, Qs * 512 + 64 * c: Qs * 512 + 64 * (c + 1)] = r[Qs, b]
    return out_full
